# revision 33
# baseline (speedup 1.0000x reference)
"""Trainium2 Bass kernel for nn_BAR_86045374808446 (sparse_attention).

Math per head h (one head per NeuronCore, 8 cores):
  s[i,j,d] = ahat_i[d] + bhat_j[d]          (ahat/bhat are d-mean-centered)
  var[i,j] = va[i] + vb[j] + (2/D)<ahat_i, bhat_j>     (one PE matmul per block)
  r[i,j]   = 1/sqrt(var + eps)
  out[i,d] = sum_{j<=i} exp(s[i,j,d] * r[i,j])

Factorization (polynomial P(x) ~ exp(x) on the observed x-range):
  exp(s*r) = exp(ahat*rbar) * exp(bhat*rbar) * exp(s*w),  w = r - rbar
  exp(s*w) ~ P(s*w) = sum_k c_k (s*w)^k
  (s*w)^k  = sum_{p+e=k} k!/(p!e!) ahat^p bhat^e w^k
  => out = sum_p A_p (*) sum_k (M*w^k)^T @ (d_k * B_{k-p}),  d_k = c_k k!
  with A_p = ahat^p/p! * exp(ahat*rbar)  [i,d],
       B_e = bhat^e/e! * exp(bhat*rbar)  [j,d],
  so the T^2*D work is bf16 PSUM-accumulated matmuls on the TensorEngine,
  and the polynomial coefficients ride on pre-scaled bf16 rhs copies (B2).
  rbar = 1/sqrt(mean va + mean vb + eps) -- picked to center the x-range;
  c_k are a Chebyshev fit of exp on that range (error budget 2e-2 rel).
  The var matmul uses RAW b rows: <ahat, bhat> = <ahat, b> since ahat is
  centered, so the b-side transpose never waits for centering.
"""

import math
import sys

import numpy as np

for _p in ("/opt/trn_rl_repo", "/root/.axon_site/_ro/trn_rl_repo"):
    if _p not in sys.path:
        sys.path.insert(0, _p)

T, D, H, P, NB = 512, 64, 8, 128, 4
EPS = 1e-5
DEG = 4
COEF = {
    4: [0.99963261, 0.99058825, 0.50079216, 0.18677153, 0.043321831],
    5: [1.00029, 0.99982237, 0.49719599, 0.16689019, 0.045660714,
        0.0085691588],
}

_cached = {}


def _build_nc(deg=DEG, dump=None):
    import concourse.bass as bass
    import concourse.mybir as mybir
    from concourse.tile import TileContext
    from concourse.masks import make_identity

    f32 = mybir.dt.float32
    f32r = mybir.dt.float32r
    bf16 = mybir.dt.bfloat16
    Alu = mybir.AluOpType
    Act = mybir.ActivationFunctionType

    coef = COEF[deg]
    dk = [float(coef[k]) * math.factorial(k) for k in range(deg + 1)]
    CHUNK = (deg + 1) * D

    nc = bass.Bass()
    ah_d = nc.declare_dram_parameter("ah", [T, D], f32, isOutput=False)
    bh_d = nc.declare_dram_parameter("bh", [T, D], f32, isOutput=False)
    out_d = nc.declare_dram_parameter("out", [T, D], f32, isOutput=True)
    dbg_d = (nc.declare_dram_parameter("dbg", [P, 4 * T], f32, isOutput=True)
             if dump else None)

    with TileContext(nc) as tc:
        with (
            tc.tile_pool(name="const", bufs=1) as constp,
            tc.tile_pool(name="work", bufs=1) as work,
            tc.tile_pool(name="wpool", bufs=12) as wpool,
            tc.tile_pool(name="w1pool", bufs=4) as w1pool,
            tc.tile_pool(name="rpool", bufs=4) as rpool,
            tc.tile_pool(name="fin", bufs=4) as fin,
            tc.tile_pool(name="psum", bufs=1, space="PSUM") as psum,
            tc.tile_pool(name="psumR", bufs=1, space="PSUM") as psumR,
        ):
            # ------- loads on two queues (SP / Act) -------------------------
            Asb = work.tile([P, NB, D], f32, tag="Asb")
            Bsb = work.tile([P, NB, D], f32, tag="Bsb")
            nc.sync.dma_start(out=Asb,
                              in_=ah_d[:].rearrange("(nb p) d -> p nb d", p=P))
            nc.scalar.dma_start(out=Bsb,
                                in_=bh_d[:].rearrange("(nb p) d -> p nb d", p=P))

            # ------- constants (identity first: gates the PE warm-up) -------
            identity = constp.tile([P, P], f32, tag="ident")
            make_identity(nc, identity)
            eps_col = constp.tile([P, 1], f32, tag="eps")
            nc.vector.memset(eps_col, EPS)
            ones1p = constp.tile([1, P], f32, tag="ones1p")
            nc.vector.memset(ones1p, 1.0)
            ones_col = constp.tile([P, 1], f32, tag="ones_col")
            nc.vector.memset(ones_col, 1.0)
            ones_bf = constp.tile([P, T], bf16, tag="ones_bf")
            nc.gpsimd.memset(ones_bf, 1.0)
            # causal mask (j<=i within-block pattern; same for every m)
            mask0 = constp.tile([P, T], bf16, tag="mask0")
            nc.gpsimd.affine_select(
                out=mask0, in_=ones_bf, compare_op=Alu.is_ge, fill=0.0,
                base=0, channel_multiplier=-1, pattern=[[1, T]])
            # zero-padded full-width rhs slab for the m=0,k=0 start matmul
            B2p0 = work.tile([P, NB, CHUNK], bf16, tag="B2p0")
            nc.gpsimd.memset(B2p0[:, :, D:], 0.0)
            warm = constp.tile([P, 1], f32, tag="warm")
            nc.scalar.activation(out=warm, in_=eps_col, func=Act.Sqrt)
            nc.scalar.activation(out=warm, in_=eps_col, func=Act.Exp)

            # PSUM: 4 accum banks + 2 transpose scratch + rbar bank
            scratch = psum.tile([P, 512], f32, tag="scratch")
            scratch2 = psum.tile([P, 512], f32, tag="scratch2")
            rbp = psumR.tile([P, 8], f32, tag="rbp")

            # PE pstate warm-up: back-to-back identity transposes
            for i in range(8):
                tp = scratch[:, (i % 4) * P:(i % 4) * P + P]
                nc.tensor.transpose(tp, identity, identity)

            # ------- a-side per-block: stats -> scaled center -> transpose --
            # var[j,i] = sum_c bT[c,j]*aT[c,i] (c=0..64) + bias:
            #   rows 0:64 = b_raw (x) (2/D)*ahat, row 64 = ones (x) va;
            #   vb + eps enters through the sqrt/ln bias.
            mva = work.tile([P, NB, 2], f32, tag="mva")
            mvb = work.tile([P, NB, 2], f32, tag="mvb")
            negmua2 = work.tile([P, NB, 1], f32, tag="negmua2")
            ahat = work.tile([P, NB, D], f32, tag="ahat")
            Tsa = work.tile([P, NB, 65], f32, tag="Tsa")
            aT = work.tile([65, NB, P], f32r, tag="aT")
            bT = work.tile([65, NB, P], f32r, tag="bT")
            onesrow = constp.tile([1, NB, P], f32, tag="onesrow")
            nc.vector.memset(onesrow, 1.0)
            nc.vector.tensor_copy(out=bT[64:65, :, :], in_=onesrow)
            for blk in range(NB):
                sa = work.tile([P, 6], f32, tag="bnsA", name=f"bnsA{blk}")
                nc.vector.bn_stats(out=sa, in_=Asb[:, blk, :])
                nc.vector.bn_aggr(out=mva[:, blk, :], in_=sa)
                nc.vector.tensor_scalar(out=negmua2[:, blk, :],
                                        in0=mva[:, blk, 0:1],
                                        scalar1=-2.0 / D, scalar2=None,
                                        op0=Alu.mult)
                # Tsa = [(2/D)*ahat | va], centered+scaled in one ACT op
                nc.scalar.activation(out=Tsa[:, blk, 0:D], in_=Asb[:, blk, :],
                                     func=Act.Identity,
                                     bias=negmua2[:, blk, :], scale=2.0 / D)
                nc.vector.tensor_copy(out=Tsa[:, blk, D:D + 1],
                                      in_=mva[:, blk, 1:2])
                tp = scratch[:, blk * P:blk * P + P]
                nc.tensor.transpose(tp[0:65, :], Tsa[:, blk, :], identity)
                nc.scalar.activation(out=aT[:, blk, :],
                                     in_=tp[0:65, :], func=Act.Copy)
                nc.vector.tensor_scalar(out=ahat[:, blk, :],
                                        in0=Asb[:, blk, :],
                                        scalar1=mva[:, blk, 0:1], scalar2=None,
                                        op0=Alu.subtract)
            aT_flat = aT.rearrange("k nb p -> k (nb p)")

            # ------- b-side: raw-row transposes (no stats dependency) -------
            bhat = work.tile([P, NB, D], bf16, tag="bhat")
            vbe = work.tile([P, NB, 1], f32, tag="vbe")
            for m in range(NB):
                tp = scratch2[:, m * P:m * P + P]
                nc.tensor.transpose(tp[0:D, :], Bsb[:, m, :], identity)
                nc.vector.tensor_copy(out=bT[0:D, m, :], in_=tp[0:D, :])
            for blk in range(NB):
                sb = work.tile([P, 6], f32, tag="bnsB", name=f"bnsB{blk}")
                nc.vector.bn_stats(out=sb, in_=Bsb[:, blk, :])
                nc.vector.bn_aggr(out=mvb[:, blk, :], in_=sb)
                nc.vector.tensor_scalar(out=vbe[:, blk, :],
                                        in0=mvb[:, blk, 1:2], scalar1=EPS,
                                        scalar2=None, op0=Alu.add)
                nc.vector.tensor_scalar(out=bhat[:, blk, :],
                                        in0=Bsb[:, blk, :],
                                        scalar1=mvb[:, blk, 0:1], scalar2=None,
                                        op0=Alu.subtract)

            # ------- rbar = 1/sqrt(mean(va)+mean(vb)+eps) -------------------
            vs2 = work.tile([P, 2], f32, tag="vs2")
            nc.vector.tensor_reduce(
                out=vs2[:, 0:1],
                in_=mva[:, :, 1:2].rearrange("p nb one -> p (nb one)"),
                axis=mybir.AxisListType.X, op=Alu.add)
            nc.vector.tensor_reduce(
                out=vs2[:, 1:2],
                in_=mvb[:, :, 1:2].rearrange("p nb one -> p (nb one)"),
                axis=mybir.AxisListType.X, op=Alu.add)
            vs1 = work.tile([P, 1], f32, tag="vs1")
            nc.vector.tensor_tensor(out=vs1, in0=vs2[:, 0:1], in1=vs2[:, 1:2],
                                    op=Alu.add)
            # cross-partition sum via one matmul: [P,1]^T @ ones -> [1,1]
            nc.tensor.matmul(rbp[0:1, 1:2], vs1, ones_col, start=True,
                             stop=True, skip_group_check=True)
            u1 = work.tile([1, 1], f32, tag="u1")
            nc.scalar.activation(out=u1, in_=rbp[0:1, 1:2], func=Act.Sqrt,
                                 bias=eps_col[0:1, :], scale=1.0 / T)
            r1 = work.tile([1, 1], f32, tag="r1")
            nc.vector.reciprocal(out=r1, in_=u1)
            nc.tensor.matmul(rbp[:, 0:1], ones1p, r1, start=True, stop=True,
                             skip_group_check=True)
            rbar = work.tile([P, 1], f32, tag="rbar")
            nc.vector.tensor_copy(out=rbar, in_=rbp[:, 0:1])
            if dump == "rbar":
                nc.sync.dma_start(out=dbg_d[:, 0:1], in_=rbar)

            # ------- B side: EB + first squares; RB chain on Pool -----------
            # RB[:, :, deg-e, :] = B_e = bhat^e/e! * exp(bhat*rbar)
            RB = work.tile([P, NB, deg + 1, D], bf16, tag="RB")
            nc.scalar.activation(out=RB[:, :, deg, :], in_=bhat, func=Act.Exp,
                                 scale=rbar)
            sq2b = work.tile([P, NB, D], bf16, tag="sq2b")
            nc.scalar.activation(out=sq2b, in_=bhat, func=Act.Square,
                                 scale=math.sqrt(0.5))
            B2 = {}

            def build_b2(k):
                B2[k] = work.tile([P, NB, k + 1, D], bf16, tag=f"B2_{k}",
                                  name=f"B2_{k}")
                nc.vector.tensor_scalar(out=B2[k], in0=RB[:, :, deg - k:, :],
                                        scalar1=dk[k], scalar2=None,
                                        op0=Alu.mult)

            # head of the padded slab: d0 * B_0
            nc.vector.tensor_scalar(out=B2p0[:, :, 0:D], in0=RB[:, :, deg, :],
                                    scalar1=dk[0], scalar2=None, op0=Alu.mult)
            nc.gpsimd.tensor_tensor(out=RB[:, :, deg - 1, :], in0=bhat,
                                    in1=RB[:, :, deg, :], op=Alu.mult)
            build_b2(1)
            nc.gpsimd.tensor_tensor(out=RB[:, :, deg - 2, :], in0=sq2b,
                                    in1=RB[:, :, deg, :], op=Alu.mult)
            build_b2(2)
            if dump == "B":
                nc.sync.dma_start(
                    out=dbg_d[:, 0:(deg + 1) * NB * D // 2],
                    in_=RB.rearrange("p nb k d -> p (nb k d)").bitcast(f32))

            # ------- var matmuls -> r_m -> W chain ---------------------------
            Dt = [psum.tile([P, 512], f32, tag=f"D{ib}", name=f"D{ib}")
                  for ib in range(NB)]
            rT_all = (work.tile([P, NB, T], f32, tag="rT", name="rT")
                      if dump else None)
            Wm = []

            def emit_var(m):
                wm = T - P * m
                vp = Dt[m][:, 0:T]
                nc.tensor.matmul(vp, bT[:, m, :], aT_flat, start=True,
                                 stop=True, skip_group_check=True)
                rt = rpool.tile([P, T], f32, tag="rt", name=f"r{m}")
                ut = rpool.tile([P, T], f32, tag="ut", name=f"u{m}")
                # keep DVE free: r = exp(-0.5*ln(v+vb+eps)) entirely on ACT
                nc.scalar.activation(out=ut, in_=vp, func=Act.Ln,
                                     bias=vbe[:, m, :], scale=1.0)
                nc.scalar.activation(out=rt, in_=ut, func=Act.Exp,
                                     scale=-0.5)
                if dump:
                    nc.vector.tensor_copy(out=rT_all[:, m, :], in_=rt)
                # W_1 = mask*(r - rbar); higher powers by pair products,
                # build order 2,3,4[,5] matching the ascending consumers.
                W1 = w1pool.tile([P, T], bf16, tag="W1", name=f"W1_{m}")
                nc.vector.scalar_tensor_tensor(
                    out=W1[:, 0:wm], in0=rt[:, P * m:T], scalar=rbar,
                    in1=mask0[:, 0:wm], op0=Alu.subtract, op1=Alu.mult)
                W = {0: mask0, 1: W1}
                build = [(2, 1, 1), (3, 1, 2), (4, 2, 2)]
                if deg == 5:
                    build.append((5, 2, 3))
                for k, a_, b_ in build[:deg - 1]:
                    W[k] = wpool.tile([P, T], bf16, tag="W", name=f"W{k}_{m}")
                    nc.vector.tensor_tensor(
                        out=W[k][:, 0:wm], in0=W[a_][:, 0:wm],
                        in1=W[b_][:, 0:wm], op=Alu.mult)
                Wm.append(W)

            emit_var(0)
            emit_var(1)
            # higher RB slots: needed from pass-0 k=3 onward, built while the
            # m=0/1 W chains occupy DVE
            sq3b = work.tile([P, NB, D], bf16, tag="sq3b")
            nc.scalar.activation(out=sq3b, in_=bhat, func=Act.Square,
                                 scale=math.sqrt(1.0 / 6))
            if deg >= 3:
                nc.gpsimd.tensor_tensor(out=RB[:, :, deg - 3, :], in0=sq3b,
                                        in1=RB[:, :, deg - 1, :], op=Alu.mult)
                build_b2(3)
            sq6b = work.tile([P, NB, D], bf16, tag="sq6b")
            nc.scalar.activation(out=sq6b, in_=bhat, func=Act.Square,
                                 scale=math.sqrt(1.0 / 12))
            if deg >= 4:
                nc.gpsimd.tensor_tensor(out=RB[:, :, deg - 4, :], in0=sq6b,
                                        in1=RB[:, :, deg - 2, :], op=Alu.mult)
                build_b2(4)
            emit_var(2)
            # ------- A side, first half (EA, A1, A2) ------------------------
            A_all = work.tile([P, NB, deg + 1, D], f32, tag="A_all")
            nc.scalar.activation(out=A_all[:, :, 0, :], in_=ahat, func=Act.Exp,
                                 scale=rbar)
            sq2a = work.tile([P, NB, D], f32, tag="sq2a")
            nc.scalar.activation(out=sq2a, in_=ahat, func=Act.Square,
                                 scale=math.sqrt(0.5))
            nc.gpsimd.tensor_tensor(out=A_all[:, :, 1, :], in0=ahat,
                                    in1=A_all[:, :, 0, :], op=Alu.mult)
            nc.vector.tensor_tensor(out=A_all[:, :, 2, :], in0=sq2a,
                                    in1=A_all[:, :, 0, :], op=Alu.mult)
            emit_var(3)

            # ------- A side, second half (A3, A4) ---------------------------
            sq3a = work.tile([P, NB, D], f32, tag="sq3a")
            nc.scalar.activation(out=sq3a, in_=ahat, func=Act.Square,
                                 scale=math.sqrt(1.0 / 6))
            sq6a = work.tile([P, NB, D], f32, tag="sq6a")
            nc.scalar.activation(out=sq6a, in_=ahat, func=Act.Square,
                                 scale=math.sqrt(1.0 / 12))
            if deg >= 3:
                nc.gpsimd.tensor_tensor(out=A_all[:, :, 3, :], in0=sq3a,
                                        in1=A_all[:, :, 1, :], op=Alu.mult)
            if deg >= 4:
                nc.vector.tensor_tensor(out=A_all[:, :, 4, :], in0=sq6a,
                                        in1=A_all[:, :, 2, :], op=Alu.mult)
            if deg >= 5:
                nc.vector.scalar_tensor_tensor(
                    out=A_all[:, :, 5, :], in0=sq2a, scalar=1.0 / 10,
                    in1=A_all[:, :, 3, :], op0=Alu.mult, op1=Alu.mult)
            if dump == "A":
                nc.sync.dma_start(
                    out=dbg_d[:, 0:(deg + 1) * NB * D],
                    in_=A_all.rearrange("p nb k d -> p (nb k d)"))

            # ------- main accumulation passes (m-major, k ascending) --------
            Wdump = (work.tile([P, 4, T], f32, tag="Wdump", name="Wdump")
                     if dump == "W" else None)

            osb = work.tile([P, NB, D], f32, tag="osb")

            def emit_final(m):
                tmp = fin.tile([P, CHUNK], f32, tag="tmp", name=f"tmp{m}")
                nc.vector.tensor_tensor(out=tmp, in0=A_all[:, m, :, :],
                                        in1=Dt[m][:, 0:CHUNK], op=Alu.mult)
                if m < NB - 1:
                    # off the critical path: binary add tree on idle Pool
                    t3 = fin.tile([P, 2, D], f32, tag="t3", name=f"t3_{m}")
                    nc.gpsimd.tensor_tensor(
                        out=t3, in0=tmp.rearrange("p (s d) -> p s d", s=deg + 1)[:, 0:2, :],
                        in1=tmp.rearrange("p (s d) -> p s d", s=deg + 1)[:, 2:4, :],
                        op=Alu.add)
                    nc.gpsimd.tensor_tensor(out=t3[:, 0, :], in0=t3[:, 0, :],
                                            in1=t3[:, 1, :], op=Alu.add)
                    nc.gpsimd.tensor_tensor(out=osb[:, m, :], in0=t3[:, 0, :],
                                            in1=tmp[:, deg * D:(deg + 1) * D],
                                            op=Alu.add)
                    if m == NB - 2:
                        # ship blocks 0..2 while block 3 finishes
                        nc.scalar.dma_start(
                            out=out_d[0:(NB - 1) * P, :].rearrange(
                                "(nb p) d -> p nb d", p=P),
                            in_=osb[:, 0:NB - 1, :])
                else:
                    nc.vector.tensor_reduce(
                        out=osb[:, m, :],
                        in_=tmp.rearrange("p (s d) -> p d s", s=deg + 1),
                        axis=mybir.AxisListType.X, op=Alu.add)
                    nc.sync.dma_start(
                        out=out_d[(NB - 1) * P:NB * P, :], in_=osb[:, m, :])

            for m in range(NB):
                W = Wm[m]
                for k in range(deg + 1):
                    for ib in range(m, NB):
                        lhsT = W[k][:, (ib - m) * P:(ib - m) * P + P]
                        last = (m == ib and k == deg)
                        if m == 0 and k == 0:
                            nc.tensor.matmul(Dt[ib][:, 0:CHUNK], lhsT,
                                             B2p0[:, 0, :], start=True,
                                             stop=last, skip_group_check=True)
                        elif k == 0:
                            nc.tensor.matmul(Dt[ib][:, 0:D], lhsT,
                                             B2p0[:, m, 0:D], start=False,
                                             stop=last, skip_group_check=True)
                        else:
                            nc.tensor.matmul(Dt[ib][:, 0:(k + 1) * D], lhsT,
                                             B2[k][:, m, :, :], start=False,
                                             stop=last, skip_group_check=True)
                if dump == "W" and m == 0:
                    for k in range(1, min(deg + 1, 5)):
                        nc.vector.tensor_copy(out=Wdump[:, k - 1, :],
                                              in_=W[k][:, 0:T])
                    nc.sync.dma_start(out=dbg_d[:], in_=Wdump.rearrange(
                        "p f t -> p (f t)"))
                emit_final(m)

            if dump == "r":
                nc.sync.dma_start(out=dbg_d[:], in_=rT_all.rearrange(
                    "p nb t -> p (nb t)"))
            if dump == "D":
                for ib in range(2):
                    dcp = fin.tile([P, CHUNK], f32, tag="dcp", name=f"dcp{ib}")
                    nc.vector.tensor_copy(out=dcp, in_=Dt[ib][:, 0:CHUNK])
                    nc.sync.dma_start(out=dbg_d[:, ib * CHUNK:(ib + 1) * CHUNK],
                                      in_=dcp)

    _split_multi_waits(nc, mybir)
    return nc


def _split_multi_waits(nc, mybir):
    """TRN2 TPB instructions have a single sync-wait slot; walrus cannot
    split >1 wait for several structs. Use the bacc rust pass to split
    them into EventSemaphore instructions."""
    import bass_rust as _bass_rust
    _bass_rust.generate_event_semaphores(nc)
    # walrus rejects wait-only EventSemaphore encodings ("ISA wrong length")
    # and requires update_value == 1. Give each wait-carrier a +1 update of a
    # scratch semaphore nothing ever waits on.
    used = set()
    for f in nc.m.functions:
        for blk in f.blocks:
            for inst in blk.instructions:
                si = getattr(inst, "sync_info", None)
                if si is not None:
                    for w in (si.on_wait or []):
                        used.add(w.id)
                    for u in (si.on_update or []):
                        used.add(u.id)
    scratch = next(s for s in nc._kernel_sem_range if s not in used)
    for f in nc.m.functions:
        for blk in f.blocks:
            for inst in blk.instructions:
                if isinstance(inst, mybir.InstEventSemaphore):
                    si = inst.sync_info
                    if si is not None and si.on_wait and not si.on_update:
                        si.on_update = [_bass_rust.SyncUpdate(
                            sync_type='semaphore', id=scratch,
                            ant_name='wsplit_scratch',
                            update_mode='sem-inc', update_value=1,
                            update_reg=None)]
    # Drop end-of-kernel EVENT_SEMAPHORE_RANGE_CLEAR (opcode 0xb0): this
    # walrus build rejects its encoding ("ISA wrong length"), and the kernel
    # preamble re-clears all kernel semaphores on every run anyway.
    for f in nc.m.functions:
        for blk in f.blocks:
            blk.instructions[:] = [
                inst for inst in blk.instructions
                if not (isinstance(inst, mybir.InstISA)
                        and getattr(inst, "isa_opcode", None) == 0xb0
                        and not (inst.sync_info and
                                 (inst.sync_info.on_wait or
                                  inst.sync_info.on_update)))
            ]


def _get_nc(deg=DEG, dump=None):
    key = ("nc", deg, dump)
    if key not in _cached:
        _cached[key] = _build_nc(deg, dump)
    return _cached[key]


def kernel(a, b, num_head=8, head_size=64, **kwargs):
    from concourse.bass_utils import run_bass_kernel_spmd

    a = np.asarray(a)
    b = np.asarray(b)
    nc = _get_nc()
    in_maps = []
    for h in range(H):
        in_maps.append({
            "ah": np.ascontiguousarray(a[0, :, h * D:(h + 1) * D], dtype=np.float32),
            "bh": np.ascontiguousarray(b[0, :, h * D:(h + 1) * D], dtype=np.float32),
        })
    res = run_bass_kernel_spmd(nc, in_maps, list(range(H)))
    full = np.concatenate([res.results[h]["out"] for h in range(H)], axis=-1)
    return full[None].astype(np.float32)


if __name__ == "__main__":
    sys.path.insert(0, "/opt/trn_rl_repo")
    _build_nc()
    print("build OK")


# revision 34
# speedup vs baseline: 1.0008x; 1.0008x over previous
"""Trainium2 Bass kernel for nn_BAR_86045374808446 (sparse_attention).

Math per head h (one head per NeuronCore, 8 cores):
  s[i,j,d] = ahat_i[d] + bhat_j[d]          (ahat/bhat are d-mean-centered)
  var[i,j] = va[i] + vb[j] + (2/D)<ahat_i, bhat_j>     (one PE matmul per block)
  r[i,j]   = 1/sqrt(var + eps)
  out[i,d] = sum_{j<=i} exp(s[i,j,d] * r[i,j])

Factorization (polynomial P(x) ~ exp(x) on the observed x-range):
  exp(s*r) = exp(ahat*rbar) * exp(bhat*rbar) * exp(s*w),  w = r - rbar
  exp(s*w) ~ P(s*w) = sum_k c_k (s*w)^k
  (s*w)^k  = sum_{p+e=k} k!/(p!e!) ahat^p bhat^e w^k
  => out = sum_p A_p (*) sum_k (M*w^k)^T @ (d_k * B_{k-p}),  d_k = c_k k!
  with A_p = ahat^p/p! * exp(ahat*rbar)  [i,d],
       B_e = bhat^e/e! * exp(bhat*rbar)  [j,d],
  so the T^2*D work is bf16 PSUM-accumulated matmuls on the TensorEngine,
  and the polynomial coefficients ride on pre-scaled bf16 rhs copies (B2).
  rbar = 1/sqrt(mean va + mean vb + eps) -- picked to center the x-range;
  c_k are a Chebyshev fit of exp on that range (error budget 2e-2 rel).
  The var matmul uses RAW b rows: <ahat, bhat> = <ahat, b> since ahat is
  centered, so the b-side transpose never waits for centering.
"""

import math
import sys

import numpy as np

for _p in ("/opt/trn_rl_repo", "/root/.axon_site/_ro/trn_rl_repo"):
    if _p not in sys.path:
        sys.path.insert(0, _p)

T, D, H, P, NB = 512, 64, 8, 128, 4
EPS = 1e-5
DEG = 4
COEF = {
    4: [0.99963261, 0.99058825, 0.50079216, 0.18677153, 0.043321831],
    5: [1.00029, 0.99982237, 0.49719599, 0.16689019, 0.045660714,
        0.0085691588],
}

_cached = {}


def _build_nc(deg=DEG, dump=None):
    import concourse.bass as bass
    import concourse.mybir as mybir
    from concourse.tile import TileContext
    from concourse.masks import make_identity

    f32 = mybir.dt.float32
    f32r = mybir.dt.float32r
    bf16 = mybir.dt.bfloat16
    Alu = mybir.AluOpType
    Act = mybir.ActivationFunctionType

    coef = COEF[deg]
    dk = [float(coef[k]) * math.factorial(k) for k in range(deg + 1)]
    CHUNK = (deg + 1) * D

    nc = bass.Bass()
    ah_d = nc.declare_dram_parameter("ah", [T, D], f32, isOutput=False)
    bh_d = nc.declare_dram_parameter("bh", [T, D], f32, isOutput=False)
    out_d = nc.declare_dram_parameter("out", [T, D], f32, isOutput=True)
    dbg_d = (nc.declare_dram_parameter("dbg", [P, 4 * T], f32, isOutput=True)
             if dump else None)

    with TileContext(nc) as tc:
        with (
            tc.tile_pool(name="const", bufs=1) as constp,
            tc.tile_pool(name="work", bufs=1) as work,
            tc.tile_pool(name="wpool", bufs=12) as wpool,
            tc.tile_pool(name="w1pool", bufs=4) as w1pool,
            tc.tile_pool(name="rpool", bufs=4) as rpool,
            tc.tile_pool(name="fin", bufs=4) as fin,
            tc.tile_pool(name="psum", bufs=1, space="PSUM") as psum,
            tc.tile_pool(name="psumR", bufs=1, space="PSUM") as psumR,
        ):
            # ------- loads on two queues (SP / Act) -------------------------
            Asb = work.tile([P, NB, D], f32, tag="Asb")
            Bsb = work.tile([P, NB, D], f32, tag="Bsb")
            nc.sync.dma_start(out=Asb,
                              in_=ah_d[:].rearrange("(nb p) d -> p nb d", p=P))
            nc.scalar.dma_start(out=Bsb,
                                in_=bh_d[:].rearrange("(nb p) d -> p nb d", p=P))

            # ------- constants (identity first: gates the PE warm-up) -------
            identity = constp.tile([P, P], f32, tag="ident")
            make_identity(nc, identity)
            eps_col = constp.tile([P, 1], f32, tag="eps")
            nc.vector.memset(eps_col, EPS)
            ones1p = constp.tile([1, P], f32, tag="ones1p")
            nc.vector.memset(ones1p, 1.0)
            ones_col = constp.tile([P, 1], f32, tag="ones_col")
            nc.vector.memset(ones_col, 1.0)
            ones_bf = constp.tile([P, T], bf16, tag="ones_bf")
            nc.gpsimd.memset(ones_bf, 1.0)
            # causal mask (j<=i within-block pattern; same for every m)
            mask0 = constp.tile([P, T], bf16, tag="mask0")
            nc.gpsimd.affine_select(
                out=mask0, in_=ones_bf, compare_op=Alu.is_ge, fill=0.0,
                base=0, channel_multiplier=-1, pattern=[[1, T]])
            # zero-padded full-width rhs slab for the m=0,k=0 start matmul
            B2p0 = work.tile([P, NB, CHUNK], bf16, tag="B2p0")
            nc.gpsimd.memset(B2p0[:, :, D:], 0.0)
            warm = constp.tile([P, 1], f32, tag="warm")
            nc.scalar.activation(out=warm, in_=eps_col, func=Act.Sqrt)
            nc.scalar.activation(out=warm, in_=eps_col, func=Act.Exp)

            # PSUM: 4 accum banks + 2 transpose scratch + rbar bank
            scratch = psum.tile([P, 512], f32, tag="scratch")
            scratch2 = psum.tile([P, 512], f32, tag="scratch2")
            rbp = psumR.tile([P, 8], f32, tag="rbp")

            # PE pstate warm-up: back-to-back identity transposes
            for i in range(8):
                tp = scratch[:, (i % 4) * P:(i % 4) * P + P]
                nc.tensor.transpose(tp, identity, identity)

            # ------- a-side per-block: stats -> scaled center -> transpose --
            # var[j,i] = sum_c bT[c,j]*aT[c,i] (c=0..64) + bias:
            #   rows 0:64 = b_raw (x) (2/D)*ahat, row 64 = ones (x) va;
            #   vb + eps enters through the sqrt/ln bias.
            mva = work.tile([P, NB, 2], f32, tag="mva")
            mvb = work.tile([P, NB, 2], f32, tag="mvb")
            negmua2 = work.tile([P, NB, 1], f32, tag="negmua2")
            ahat = work.tile([P, NB, D], f32, tag="ahat")
            Tsa = work.tile([P, NB, 65], f32, tag="Tsa")
            aT = work.tile([65, NB, P], f32r, tag="aT")
            bT = work.tile([65, NB, P], f32r, tag="bT")
            onesrow = constp.tile([1, NB, P], f32, tag="onesrow")
            nc.vector.memset(onesrow, 1.0)
            nc.vector.tensor_copy(out=bT[64:65, :, :], in_=onesrow)
            for blk in range(NB):
                sa = work.tile([P, 6], f32, tag="bnsA", name=f"bnsA{blk}")
                nc.vector.bn_stats(out=sa, in_=Asb[:, blk, :])
                nc.vector.bn_aggr(out=mva[:, blk, :], in_=sa)
                nc.vector.tensor_scalar(out=negmua2[:, blk, :],
                                        in0=mva[:, blk, 0:1],
                                        scalar1=-2.0 / D, scalar2=None,
                                        op0=Alu.mult)
                # Tsa = [(2/D)*ahat | va], centered+scaled in one ACT op
                nc.scalar.activation(out=Tsa[:, blk, 0:D], in_=Asb[:, blk, :],
                                     func=Act.Identity,
                                     bias=negmua2[:, blk, :], scale=2.0 / D)
                nc.vector.tensor_copy(out=Tsa[:, blk, D:D + 1],
                                      in_=mva[:, blk, 1:2])
                tp = scratch[:, blk * P:blk * P + P]
                nc.tensor.transpose(tp[0:65, :], Tsa[:, blk, :], identity)
                nc.scalar.activation(out=aT[:, blk, :],
                                     in_=tp[0:65, :], func=Act.Copy)
                nc.vector.tensor_scalar(out=ahat[:, blk, :],
                                        in0=Asb[:, blk, :],
                                        scalar1=mva[:, blk, 0:1], scalar2=None,
                                        op0=Alu.subtract)
            aT_flat = aT.rearrange("k nb p -> k (nb p)")

            # ------- b-side: raw-row transposes (no stats dependency) -------
            bhat = work.tile([P, NB, D], bf16, tag="bhat")
            vbe = work.tile([P, NB, 1], f32, tag="vbe")
            for m in range(NB):
                tp = scratch2[:, m * P:m * P + P]
                nc.tensor.transpose(tp[0:D, :], Bsb[:, m, :], identity)
                nc.vector.tensor_copy(out=bT[0:D, m, :], in_=tp[0:D, :])
            for blk in range(NB):
                sb = work.tile([P, 6], f32, tag="bnsB", name=f"bnsB{blk}")
                nc.vector.bn_stats(out=sb, in_=Bsb[:, blk, :])
                nc.vector.bn_aggr(out=mvb[:, blk, :], in_=sb)
                nc.vector.tensor_scalar(out=vbe[:, blk, :],
                                        in0=mvb[:, blk, 1:2], scalar1=EPS,
                                        scalar2=None, op0=Alu.add)
                nc.vector.tensor_scalar(out=bhat[:, blk, :],
                                        in0=Bsb[:, blk, :],
                                        scalar1=mvb[:, blk, 0:1], scalar2=None,
                                        op0=Alu.subtract)

            # ------- rbar = 1/sqrt(mean(va)+mean(vb)+eps) -------------------
            vs2 = work.tile([P, 2], f32, tag="vs2")
            nc.vector.tensor_reduce(
                out=vs2[:, 0:1],
                in_=mva[:, :, 1:2].rearrange("p nb one -> p (nb one)"),
                axis=mybir.AxisListType.X, op=Alu.add)
            nc.vector.tensor_reduce(
                out=vs2[:, 1:2],
                in_=mvb[:, :, 1:2].rearrange("p nb one -> p (nb one)"),
                axis=mybir.AxisListType.X, op=Alu.add)
            vs1 = work.tile([P, 1], f32, tag="vs1")
            nc.vector.tensor_tensor(out=vs1, in0=vs2[:, 0:1], in1=vs2[:, 1:2],
                                    op=Alu.add)
            # cross-partition sum via one matmul: [P,1]^T @ ones -> [1,1]
            nc.tensor.matmul(rbp[0:1, 1:2], vs1, ones_col, start=True,
                             stop=True, skip_group_check=True)
            u1 = work.tile([1, 1], f32, tag="u1")
            nc.scalar.activation(out=u1, in_=rbp[0:1, 1:2], func=Act.Sqrt,
                                 bias=eps_col[0:1, :], scale=1.0 / T)
            r1 = work.tile([1, 1], f32, tag="r1")
            nc.vector.reciprocal(out=r1, in_=u1)
            nc.tensor.matmul(rbp[:, 0:1], ones1p, r1, start=True, stop=True,
                             skip_group_check=True)
            rbar = work.tile([P, 1], f32, tag="rbar")
            nc.vector.tensor_copy(out=rbar, in_=rbp[:, 0:1])
            if dump == "rbar":
                nc.sync.dma_start(out=dbg_d[:, 0:1], in_=rbar)

            # ------- B side: EB + first squares; RB chain on Pool -----------
            # RB[:, :, deg-e, :] = B_e = bhat^e/e! * exp(bhat*rbar)
            RB = work.tile([P, NB, deg + 1, D], bf16, tag="RB")
            nc.scalar.activation(out=RB[:, :, deg, :], in_=bhat, func=Act.Exp,
                                 scale=rbar)
            sq2b = work.tile([P, NB, D], bf16, tag="sq2b")
            nc.scalar.activation(out=sq2b, in_=bhat, func=Act.Square,
                                 scale=math.sqrt(0.5))
            B2 = {}

            def build_b2(k):
                B2[k] = work.tile([P, NB, k + 1, D], bf16, tag=f"B2_{k}",
                                  name=f"B2_{k}")
                nc.vector.tensor_scalar(out=B2[k], in0=RB[:, :, deg - k:, :],
                                        scalar1=dk[k], scalar2=None,
                                        op0=Alu.mult)

            # head of the padded slab: d0 * B_0
            nc.vector.tensor_scalar(out=B2p0[:, :, 0:D], in0=RB[:, :, deg, :],
                                    scalar1=dk[0], scalar2=None, op0=Alu.mult)
            nc.gpsimd.tensor_tensor(out=RB[:, :, deg - 1, :], in0=bhat,
                                    in1=RB[:, :, deg, :], op=Alu.mult)
            build_b2(1)
            nc.gpsimd.tensor_tensor(out=RB[:, :, deg - 2, :], in0=sq2b,
                                    in1=RB[:, :, deg, :], op=Alu.mult)
            build_b2(2)
            if dump == "B":
                nc.sync.dma_start(
                    out=dbg_d[:, 0:(deg + 1) * NB * D // 2],
                    in_=RB.rearrange("p nb k d -> p (nb k d)").bitcast(f32))

            # ------- var matmuls -> r_m -> W chain ---------------------------
            Dt = [psum.tile([P, 512], f32, tag=f"D{ib}", name=f"D{ib}")
                  for ib in range(NB)]
            rT_all = (work.tile([P, NB, T], f32, tag="rT", name="rT")
                      if dump else None)
            Wm = []

            def emit_var(m):
                wm = T - P * m
                vp = Dt[m][:, 0:T]
                nc.tensor.matmul(vp, bT[:, m, :], aT_flat, start=True,
                                 stop=True, skip_group_check=True)
                rt = rpool.tile([P, T], f32, tag="rt", name=f"r{m}")
                ut = rpool.tile([P, T], f32, tag="ut", name=f"u{m}")
                if m < 2:
                    # r = 1/sqrt(v+vb+eps): ACT sqrt + DVE reciprocal
                    nc.scalar.activation(out=ut, in_=vp, func=Act.Sqrt,
                                         bias=vbe[:, m, :], scale=1.0)
                    nc.vector.reciprocal(out=rt, in_=ut)
                else:
                    # keep DVE free: r = exp(-0.5*ln(v+vb+eps)) on ACT
                    nc.scalar.activation(out=ut, in_=vp, func=Act.Ln,
                                         bias=vbe[:, m, :], scale=1.0)
                    nc.scalar.activation(out=rt, in_=ut, func=Act.Exp,
                                         scale=-0.5)
                if dump:
                    nc.vector.tensor_copy(out=rT_all[:, m, :], in_=rt)
                # W_1 = mask*(r - rbar); higher powers by pair products,
                # build order 2,3,4[,5] matching the ascending consumers.
                W1 = w1pool.tile([P, T], bf16, tag="W1", name=f"W1_{m}")
                nc.vector.scalar_tensor_tensor(
                    out=W1[:, 0:wm], in0=rt[:, P * m:T], scalar=rbar,
                    in1=mask0[:, 0:wm], op0=Alu.subtract, op1=Alu.mult)
                W = {0: mask0, 1: W1}
                build = [(2, 1, 1), (3, 1, 2), (4, 2, 2)]
                if deg == 5:
                    build.append((5, 2, 3))
                for k, a_, b_ in build[:deg - 1]:
                    W[k] = wpool.tile([P, T], bf16, tag="W", name=f"W{k}_{m}")
                    nc.vector.tensor_tensor(
                        out=W[k][:, 0:wm], in0=W[a_][:, 0:wm],
                        in1=W[b_][:, 0:wm], op=Alu.mult)
                Wm.append(W)

            emit_var(0)
            emit_var(1)
            # higher RB slots: needed from pass-0 k=3 onward, built while the
            # m=0/1 W chains occupy DVE
            sq3b = work.tile([P, NB, D], bf16, tag="sq3b")
            nc.scalar.activation(out=sq3b, in_=bhat, func=Act.Square,
                                 scale=math.sqrt(1.0 / 6))
            if deg >= 3:
                nc.gpsimd.tensor_tensor(out=RB[:, :, deg - 3, :], in0=sq3b,
                                        in1=RB[:, :, deg - 1, :], op=Alu.mult)
                build_b2(3)
            sq6b = work.tile([P, NB, D], bf16, tag="sq6b")
            nc.scalar.activation(out=sq6b, in_=bhat, func=Act.Square,
                                 scale=math.sqrt(1.0 / 12))
            if deg >= 4:
                nc.gpsimd.tensor_tensor(out=RB[:, :, deg - 4, :], in0=sq6b,
                                        in1=RB[:, :, deg - 2, :], op=Alu.mult)
                build_b2(4)
            emit_var(2)
            emit_var(3)

            # ------- A side (consumed by the finals; can lag) ---------------
            A_all = work.tile([P, NB, deg + 1, D], f32, tag="A_all")
            nc.scalar.activation(out=A_all[:, :, 0, :], in_=ahat, func=Act.Exp,
                                 scale=rbar)
            sq2a = work.tile([P, NB, D], f32, tag="sq2a")
            nc.scalar.activation(out=sq2a, in_=ahat, func=Act.Square,
                                 scale=math.sqrt(0.5))
            sq3a = work.tile([P, NB, D], f32, tag="sq3a")
            nc.scalar.activation(out=sq3a, in_=ahat, func=Act.Square,
                                 scale=math.sqrt(1.0 / 6))
            sq6a = work.tile([P, NB, D], f32, tag="sq6a")
            nc.scalar.activation(out=sq6a, in_=ahat, func=Act.Square,
                                 scale=math.sqrt(1.0 / 12))
            nc.gpsimd.tensor_tensor(out=A_all[:, :, 1, :], in0=ahat,
                                    in1=A_all[:, :, 0, :], op=Alu.mult)
            nc.vector.tensor_tensor(out=A_all[:, :, 2, :], in0=sq2a,
                                    in1=A_all[:, :, 0, :], op=Alu.mult)
            if deg >= 3:
                nc.gpsimd.tensor_tensor(out=A_all[:, :, 3, :], in0=sq3a,
                                        in1=A_all[:, :, 1, :], op=Alu.mult)
            if deg >= 4:
                nc.vector.tensor_tensor(out=A_all[:, :, 4, :], in0=sq6a,
                                        in1=A_all[:, :, 2, :], op=Alu.mult)
            if deg >= 5:
                nc.vector.scalar_tensor_tensor(
                    out=A_all[:, :, 5, :], in0=sq2a, scalar=1.0 / 10,
                    in1=A_all[:, :, 3, :], op0=Alu.mult, op1=Alu.mult)
            if dump == "A":
                nc.sync.dma_start(
                    out=dbg_d[:, 0:(deg + 1) * NB * D],
                    in_=A_all.rearrange("p nb k d -> p (nb k d)"))

            # ------- main accumulation passes (m-major, k ascending) --------
            Wdump = (work.tile([P, 4, T], f32, tag="Wdump", name="Wdump")
                     if dump == "W" else None)

            osb = work.tile([P, NB, D], f32, tag="osb")

            def emit_final(m):
                tmp = fin.tile([P, CHUNK], f32, tag="tmp", name=f"tmp{m}")
                nc.vector.tensor_tensor(out=tmp, in0=A_all[:, m, :, :],
                                        in1=Dt[m][:, 0:CHUNK], op=Alu.mult)
                if m < NB - 1:
                    # off the critical path: binary add tree on idle Pool
                    t3 = fin.tile([P, 2, D], f32, tag="t3", name=f"t3_{m}")
                    nc.gpsimd.tensor_tensor(
                        out=t3, in0=tmp.rearrange("p (s d) -> p s d", s=deg + 1)[:, 0:2, :],
                        in1=tmp.rearrange("p (s d) -> p s d", s=deg + 1)[:, 2:4, :],
                        op=Alu.add)
                    nc.gpsimd.tensor_tensor(out=t3[:, 0, :], in0=t3[:, 0, :],
                                            in1=t3[:, 1, :], op=Alu.add)
                    nc.gpsimd.tensor_tensor(out=osb[:, m, :], in0=t3[:, 0, :],
                                            in1=tmp[:, deg * D:(deg + 1) * D],
                                            op=Alu.add)
                else:
                    nc.vector.tensor_reduce(
                        out=osb[:, m, :],
                        in_=tmp.rearrange("p (s d) -> p d s", s=deg + 1),
                        axis=mybir.AxisListType.X, op=Alu.add)
                    nc.sync.dma_start(
                        out=out_d[:].rearrange("(nb p) d -> p nb d", p=P),
                        in_=osb)

            for m in range(NB):
                W = Wm[m]
                for k in range(deg + 1):
                    for ib in range(m, NB):
                        lhsT = W[k][:, (ib - m) * P:(ib - m) * P + P]
                        last = (m == ib and k == deg)
                        if m == 0 and k == 0:
                            nc.tensor.matmul(Dt[ib][:, 0:CHUNK], lhsT,
                                             B2p0[:, 0, :], start=True,
                                             stop=last, skip_group_check=True)
                        elif k == 0:
                            nc.tensor.matmul(Dt[ib][:, 0:D], lhsT,
                                             B2p0[:, m, 0:D], start=False,
                                             stop=last, skip_group_check=True)
                        else:
                            nc.tensor.matmul(Dt[ib][:, 0:(k + 1) * D], lhsT,
                                             B2[k][:, m, :, :], start=False,
                                             stop=last, skip_group_check=True)
                if dump == "W" and m == 0:
                    for k in range(1, min(deg + 1, 5)):
                        nc.vector.tensor_copy(out=Wdump[:, k - 1, :],
                                              in_=W[k][:, 0:T])
                    nc.sync.dma_start(out=dbg_d[:], in_=Wdump.rearrange(
                        "p f t -> p (f t)"))
                emit_final(m)

            if dump == "r":
                nc.sync.dma_start(out=dbg_d[:], in_=rT_all.rearrange(
                    "p nb t -> p (nb t)"))
            if dump == "D":
                for ib in range(2):
                    dcp = fin.tile([P, CHUNK], f32, tag="dcp", name=f"dcp{ib}")
                    nc.vector.tensor_copy(out=dcp, in_=Dt[ib][:, 0:CHUNK])
                    nc.sync.dma_start(out=dbg_d[:, ib * CHUNK:(ib + 1) * CHUNK],
                                      in_=dcp)

    _split_multi_waits(nc, mybir)
    return nc


def _split_multi_waits(nc, mybir):
    """TRN2 TPB instructions have a single sync-wait slot; walrus cannot
    split >1 wait for several structs. Use the bacc rust pass to split
    them into EventSemaphore instructions."""
    import bass_rust as _bass_rust
    _bass_rust.generate_event_semaphores(nc)
    # walrus rejects wait-only EventSemaphore encodings ("ISA wrong length")
    # and requires update_value == 1. Give each wait-carrier a +1 update of a
    # scratch semaphore nothing ever waits on.
    used = set()
    for f in nc.m.functions:
        for blk in f.blocks:
            for inst in blk.instructions:
                si = getattr(inst, "sync_info", None)
                if si is not None:
                    for w in (si.on_wait or []):
                        used.add(w.id)
                    for u in (si.on_update or []):
                        used.add(u.id)
    scratch = next(s for s in nc._kernel_sem_range if s not in used)
    for f in nc.m.functions:
        for blk in f.blocks:
            for inst in blk.instructions:
                if isinstance(inst, mybir.InstEventSemaphore):
                    si = inst.sync_info
                    if si is not None and si.on_wait and not si.on_update:
                        si.on_update = [_bass_rust.SyncUpdate(
                            sync_type='semaphore', id=scratch,
                            ant_name='wsplit_scratch',
                            update_mode='sem-inc', update_value=1,
                            update_reg=None)]
    # Drop end-of-kernel EVENT_SEMAPHORE_RANGE_CLEAR (opcode 0xb0): this
    # walrus build rejects its encoding ("ISA wrong length"), and the kernel
    # preamble re-clears all kernel semaphores on every run anyway.
    for f in nc.m.functions:
        for blk in f.blocks:
            blk.instructions[:] = [
                inst for inst in blk.instructions
                if not (isinstance(inst, mybir.InstISA)
                        and getattr(inst, "isa_opcode", None) == 0xb0
                        and not (inst.sync_info and
                                 (inst.sync_info.on_wait or
                                  inst.sync_info.on_update)))
            ]


def _get_nc(deg=DEG, dump=None):
    key = ("nc", deg, dump)
    if key not in _cached:
        _cached[key] = _build_nc(deg, dump)
    return _cached[key]


def kernel(a, b, num_head=8, head_size=64, **kwargs):
    from concourse.bass_utils import run_bass_kernel_spmd

    a = np.asarray(a)
    b = np.asarray(b)
    nc = _get_nc()
    in_maps = []
    for h in range(H):
        in_maps.append({
            "ah": np.ascontiguousarray(a[0, :, h * D:(h + 1) * D], dtype=np.float32),
            "bh": np.ascontiguousarray(b[0, :, h * D:(h + 1) * D], dtype=np.float32),
        })
    res = run_bass_kernel_spmd(nc, in_maps, list(range(H)))
    full = np.concatenate([res.results[h]["out"] for h in range(H)], axis=-1)
    return full[None].astype(np.float32)


if __name__ == "__main__":
    sys.path.insert(0, "/opt/trn_rl_repo")
    _build_nc()
    print("build OK")
